# revision 1
# baseline (speedup 1.0000x reference)
"""Distributed Trainium2 kernel for the ADMM-NN fixed-point iteration:

    for _ in range(N):
        x = W @ x + b
        x[idx1:idx2] = clip(x[idx1:idx2], l, u)

Strategy (8 NeuronCores, tensor-parallel):
  - Row-shard W: core i owns 1024 rows.  The rows are PERMUTED host-side so
    that every core owns 768 un-clamped rows + 256 clamped rows; the clamp
    is then the same local slice y[768:1024] on every core (SPMD-uniform).
  - W is stored bf16, resident in SBUF (16 MB/core) -> after the initial
    load there is no HBM weight traffic at all; each iteration is a pure
    TensorEngine GEMV (x-stationary: lhsT = x k-tile [128,1], rhs = W^T
    [128,512] streaming at full rate) + two small AllGathers.
  - Software pipeline: y is produced in two chunks (psum banks A/B).
    Chunk A (y[0:512]) is all-gathered while the TensorEngine computes
    chunk B; the x-layout gamma is arranged so that k-tiles 0..31 of the
    next iteration depend only on chunk A's gather, 32..63 only on B's.
  - W's columns are pre-permuted host-side so the AllGather outputs DMA
    straight into the matmul operand layout with no transpose.

kernel(**inputs) takes the FULL unsharded inputs and returns the FULL
output, matching reference.reference().
"""

import numpy as np
import ml_dtypes

NCORES = 8
D = 8192
ROWS = D // NCORES  # 1024 rows per core
NT = 64  # contraction k-tiles of 128
HT = NT // 2  # k-tiles per chunk
P = 128  # partitions

_nc_cache = {}


def _perm(idx1, idx2):
    """Permuted order: core i owns un-clamped rows [un*i, un*(i+1)) followed
    by clamped rows [idx1 + seg*i, idx1 + seg*(i+1))."""
    assert idx2 == D and idx1 % NCORES == 0
    seg = (idx2 - idx1) // NCORES
    un = ROWS - seg
    assert un * NCORES == idx1
    parts = []
    for i in range(NCORES):
        parts.append(np.arange(un * i, un * (i + 1)))
        parts.append(np.arange(idx1 + seg * i, idx1 + seg * (i + 1)))
    return np.concatenate(parts), un


def _gamma():
    """x_sb[p, t] holds x'[gamma[p, t]].  Chunk A (t<32) covers each core's
    y[0:512] in AllGather-output flat order; chunk B (t>=32) covers
    y[512:1024]."""
    p = np.arange(P)[:, None]
    t = np.arange(NT)[None, :]
    base = (p // 16) * 1024 + (p % 16) * 32
    return np.where(t < HT, base + t, base + 512 + (t - HT))


def _build_nc(n_iter, clamp_lo, l_val, u_val):
    import concourse.bacc as bacc
    import concourse.mybir as mybir
    from concourse import tile

    assert 512 <= clamp_lo < ROWS
    cl = clamp_lo - 512
    nc = bacc.Bacc(None, target_bir_lowering=False, num_devices=NCORES)
    NWCH = 16  # W arrives as 16 chunks of 4 k-tiles each
    w_ext = [
        nc.declare_dram_parameter(
            f"W{c}", [P, (NT // NWCH) * ROWS], mybir.dt.bfloat16, isOutput=False
        )
        for c in range(NWCH)
    ]
    x0_ext = nc.declare_dram_parameter("x0", [P, NT], mybir.dt.float32, isOutput=False)
    b_ext = nc.declare_dram_parameter("bias", [1, ROWS], mybir.dt.float32, isOutput=False)
    out_ext = nc.declare_dram_parameter("out", [1, ROWS], mybir.dt.float32, isOutput=True)

    with tile.TileContext(nc) as tc:
        with (
            tc.tile_pool(name="wpool", bufs=1) as wpool,
            tc.tile_pool(name="cpool", bufs=1) as cpool,
            tc.tile_pool(name="xpool", bufs=2) as xpool,
            tc.tile_pool(name="ypool", bufs=2) as ypool,
            tc.tile_pool(name="ps", bufs=2, space="PSUM") as pspool,
            tc.tile_pool(name="dram", bufs=2, space="DRAM") as dpool,
        ):
            wt = []
            for c in range(NWCH):
                w = wpool.tile([P, (NT // NWCH) * ROWS], mybir.dt.bfloat16, tag=f"W{c}")
                nc.sync.dma_start(w[:], w_ext[c][:])
                wt.append(w)
            b_sb = cpool.tile([1, ROWS], mybir.dt.float32, tag="b")
            nc.sync.dma_start(b_sb[:], b_ext[:])

            xfA = xpool.tile([P, HT], mybir.dt.float32, tag="xfA")
            xfB = xpool.tile([P, HT], mybir.dt.float32, tag="xfB")
            nc.sync.dma_start(xfA[:], x0_ext[:, 0:HT])
            nc.sync.dma_start(xfB[:], x0_ext[:, HT:NT])
            xbA = xpool.tile([P, HT], mybir.dt.bfloat16, tag="xbA")
            xbB = xpool.tile([P, HT], mybir.dt.bfloat16, tag="xbB")
            nc.vector.tensor_copy(xbA[:], xfA[:])
            nc.vector.tensor_copy(xbB[:], xfB[:])

            def mm_loop(ps, half, xA, xB):
                for t in range(NT):
                    xb = xA if t < HT else xB
                    tt = t if t < HT else t - HT
                    c, r = t // 4, t % 4
                    nc.tensor.matmul(
                        ps[:, :],
                        xb[:, tt : tt + 1],
                        wt[c][:, r * ROWS + half * 512 : r * ROWS + half * 512 + 512],
                        start=(t == 0),
                        stop=(t == NT - 1),
                    )

            for k in range(n_iter):
                last = k == n_iter - 1
                # ---- chunk A: y[0:512] (never clamped) ----
                psA = pspool.tile([1, 512], mybir.dt.float32, tag="psA")
                mm_loop(psA, 0, xbA, xbB)
                yA = ypool.tile([1, 512], mybir.dt.float32, tag="yA")
                nc.vector.tensor_tensor(
                    yA[:, :], psA[:, :], b_sb[:, 0:512], op=mybir.AluOpType.add
                )
                xbA_next = xbA
                if last:
                    nc.sync.dma_start(out_ext[:, 0:512], yA[:])
                else:
                    aginA = dpool.tile([1, 512], mybir.dt.float32, tag="aginA")
                    agoutA = dpool.tile([P, HT], mybir.dt.float32, tag="agoutA")
                    nc.sync.dma_start(aginA[:], yA[:])
                    nc.gpsimd.collective_compute(
                        "AllGather",
                        mybir.AluOpType.bypass,
                        replica_groups=[list(range(NCORES))],
                        ins=[aginA.opt()],
                        outs=[agoutA.opt()],
                    )
                    xfA = xpool.tile([P, HT], mybir.dt.float32, tag="xfA")
                    nc.sync.dma_start(xfA[:], agoutA[:])
                    xbA_next = xpool.tile([P, HT], mybir.dt.bfloat16, tag="xbA")
                    nc.vector.tensor_copy(xbA_next[:], xfA[:])

                # ---- chunk B: y[512:1024] (clamp on [clamp_lo:1024)) ----
                psB = pspool.tile([1, 512], mybir.dt.float32, tag="psB")
                mm_loop(psB, 1, xbA, xbB)
                yB = ypool.tile([1, 512], mybir.dt.float32, tag="yB")
                nc.vector.tensor_tensor(
                    yB[:, :], psB[:, :], b_sb[:, 512:1024], op=mybir.AluOpType.add
                )
                nc.vector.tensor_scalar(
                    yB[:, cl:512],
                    yB[:, cl:512],
                    float(l_val),
                    float(u_val),
                    mybir.AluOpType.max,
                    mybir.AluOpType.min,
                )
                if last:
                    nc.sync.dma_start(out_ext[:, 512:1024], yB[:])
                else:
                    aginB = dpool.tile([1, 512], mybir.dt.float32, tag="aginB")
                    agoutB = dpool.tile([P, HT], mybir.dt.float32, tag="agoutB")
                    nc.sync.dma_start(aginB[:], yB[:])
                    nc.gpsimd.collective_compute(
                        "AllGather",
                        mybir.AluOpType.bypass,
                        replica_groups=[list(range(NCORES))],
                        ins=[aginB.opt()],
                        outs=[agoutB.opt()],
                    )
                    xfB = xpool.tile([P, HT], mybir.dt.float32, tag="xfB")
                    nc.sync.dma_start(xfB[:], agoutB[:])
                    xbB = xpool.tile([P, HT], mybir.dt.bfloat16, tag="xbB")
                    nc.vector.tensor_copy(xbB[:], xfB[:])
                xbA = xbA_next
    nc.compile()
    return nc


def _get_nc(n_iter, clamp_lo, l_val, u_val):
    key = (n_iter, clamp_lo, float(l_val), float(u_val))
    if key not in _nc_cache:
        _nc_cache[key] = _build_nc(n_iter, clamp_lo, l_val, u_val)
    return _nc_cache[key]


def _prep_in_maps(x, W, b, idx1, idx2):
    perm, _un = _perm(idx1, idx2)
    g = _gamma()
    colidx = perm[g]  # [128, 64] original column index per (p, t)
    xp = np.asarray(x, np.float32)[perm]
    bp = np.asarray(b, np.float32)[perm]
    x0_layout = np.ascontiguousarray(xp[g], np.float32)
    bf16 = ml_dtypes.bfloat16
    NWCH = 16
    in_maps = []
    for i in range(NCORES):
        rows_i = perm[ROWS * i : ROWS * (i + 1)]
        Wi = W[rows_i]  # [1024, 8192]
        Wc = Wi[:, colidx.reshape(-1)].reshape(ROWS, P, NT)  # [n, p, t]
        Wt = np.ascontiguousarray(
            np.transpose(Wc, (1, 2, 0)).reshape(P, NT * ROWS)
        ).astype(bf16)  # Wt[p, t*1024 + n]
        m = {
            f"W{c}": np.ascontiguousarray(
                Wt[:, c * (NT // NWCH) * ROWS : (c + 1) * (NT // NWCH) * ROWS]
            )
            for c in range(NWCH)
        }
        m["x0"] = x0_layout
        m["bias"] = np.ascontiguousarray(bp[ROWS * i : ROWS * (i + 1)].reshape(1, ROWS))
        in_maps.append(m)
    return in_maps, perm


def run(x, W, b, l, u, idx1, idx2, N, trace=False, trace_kwargs=None):
    from concourse.bass_utils import run_bass_kernel_spmd

    x = np.asarray(x, np.float32)
    W = np.asarray(W, np.float32)
    b = np.asarray(b, np.float32)
    l = float(np.asarray(l))
    u = float(np.asarray(u))
    idx1 = int(np.asarray(idx1))
    idx2 = int(np.asarray(idx2))
    N = int(np.asarray(N))
    assert x.shape == (D,) and W.shape == (D, D) and b.shape == (D,)
    assert N >= 1

    seg = (idx2 - idx1) // NCORES
    clamp_lo = ROWS - seg
    nc = _get_nc(N, clamp_lo, l, u)
    in_maps, perm = _prep_in_maps(x, W, b, idx1, idx2)
    res = run_bass_kernel_spmd(
        nc,
        in_maps,
        core_ids=list(range(NCORES)),
        trace=trace,
        **(trace_kwargs or {}),
    )
    chunks = [np.asarray(res.results[i]["out"], np.float32).reshape(ROWS) for i in range(NCORES)]
    xp_final = np.concatenate(chunks)
    out = np.empty(D, np.float32)
    out[perm] = xp_final
    return out, res


def kernel(**inputs):
    out, _ = run(
        inputs["x"],
        inputs["W"],
        inputs["b"],
        inputs["l"],
        inputs["u"],
        inputs["idx1"],
        inputs["idx2"],
        inputs["N"],
        trace=False,
    )
    return out



# revision 2
# speedup vs baseline: 1.4150x; 1.4150x over previous
"""Distributed Trainium2 kernel for the ADMM-NN fixed-point iteration:

    for _ in range(N):
        x = W @ x + b
        x[idx1:idx2] = clip(x[idx1:idx2], l, u)

v2: 4-way column-tiled TensorE GEMV (x-stationary), one bf16 AllGather
per iteration.

  - Row-shard W: core i owns 1024 rows, permuted so local y layout is
    j = g*256 + n with group g in {0..3}; group 3 is the clamp segment.
  - W resident in SBUF as bf16 [128, 64*4*256] (16 MB/core).
  - Per k-tile t: 4 concurrent matmuls (tile_position=(0,32g)), each
    [128,1] x-column stationary x [128,256] W moving -> psum row 32g.
    4 independent moving streams ~= 4x the single-stream W bandwidth.
  - DVE: psum + bias -> bf16 y tile; clamp group 3; 4 row-DMAs to a
    DRAM bounce; AllGather (bf16, 2KB/core in); DMA gathered [128,64]
    straight into the next iteration's stationary x tile (the AG
    partition-major layout IS the stationary layout by construction of
    the host-side W column permutation).

kernel(**inputs) takes FULL unsharded inputs, returns the FULL output.
"""

import numpy as np
import ml_dtypes

NCORES = 8
D = 8192
ROWS = D // NCORES  # 1024
NT = 64             # k-tiles of 128
P = 128
NG = 4              # column-tile groups
GW = ROWS // NG     # 256 outputs per group
NWCH = 16           # W arrives as 16 dram params

_nc_cache = {}


def _perm(idx1, idx2):
    """perm[i*1024 + j] = global row owned by core i at local position j,
    with local layout j = g*256 + n and group 3 = the clamp rows."""
    assert idx2 == D and idx1 == 6144
    un = ROWS - (idx2 - idx1) // NCORES  # 768 unclamped rows per core
    parts = []
    for i in range(NCORES):
        parts.append(np.arange(un * i, un * (i + 1)))
        parts.append(idx1 + np.arange(256 * i, 256 * (i + 1)))
    return np.concatenate(parts), un


def _colmap():
    """colmap[p, t] = global x index feeding stationary cell (p, t).
    AllGather output layout: flat[c*1024 + j] (c-major, local order j),
    viewed as [128, 64] partition-major: x_sb[p, t] = flat[p*64 + t]."""
    p = np.arange(P)[:, None]
    t = np.arange(NT)[None, :]
    return (p * NT + t)  # index into flat local-order concat; perm applied later


def _build_nc(n_iter, l_val, u_val):
    import concourse.bacc as bacc
    import concourse.mybir as mybir
    from concourse import tile

    nc = bacc.Bacc(None, target_bir_lowering=False, num_devices=NCORES)
    wcols = NT * NG * GW  # 65536
    w_ext = [
        nc.declare_dram_parameter(
            f"W{c}", [P, wcols // NWCH], mybir.dt.bfloat16, isOutput=False
        )
        for c in range(NWCH)
    ]
    x0_ext = nc.declare_dram_parameter("x0", [P, NT], mybir.dt.float32, isOutput=False)
    b_ext = nc.declare_dram_parameter("bias", [P, GW], mybir.dt.float32, isOutput=False)
    out_ext = nc.declare_dram_parameter("out", [1, ROWS], mybir.dt.float32, isOutput=True)

    with tile.TileContext(nc) as tc:
        with (
            tc.tile_pool(name="wpool", bufs=1) as wpool,
            tc.tile_pool(name="cpool", bufs=1) as cpool,
            tc.tile_pool(name="xpool", bufs=2) as xpool,
            tc.tile_pool(name="ypool", bufs=2) as ypool,
            tc.tile_pool(name="ps", bufs=2, space="PSUM") as pspool,
            tc.tile_pool(name="dram", bufs=2, space="DRAM") as dpool,
        ):
            wt = []
            for c in range(NWCH):
                w = wpool.tile([P, wcols // NWCH], mybir.dt.bfloat16, tag=f"W{c}")
                nc.sync.dma_start(w[:], w_ext[c][:])
                wt.append(w)
            b_sb = cpool.tile([P, GW], mybir.dt.float32, tag="b")
            nc.sync.dma_start(b_sb[:], b_ext[:])

            xf = xpool.tile([P, NT], mybir.dt.float32, tag="xf")
            nc.sync.dma_start(xf[:], x0_ext[:])
            xb = xpool.tile([P, NT], mybir.dt.bfloat16, tag="xb")
            nc.vector.tensor_copy(xb[:], xf[:])

            def wblock(t, g):
                col = (t * NG + g) * GW
                c, off = divmod(col, wcols // NWCH)
                return wt[c][:, off : off + GW]

            for k in range(n_iter):
                last = k == n_iter - 1
                ps = pspool.tile([P, GW], mybir.dt.float32, tag="ps")
                for t in range(NT):
                    for g in range(NG):
                        nc.tensor.matmul(
                            ps[32 * g : 32 * g + 1, :],
                            xb[:, t : t + 1],
                            wblock(t, g),
                            start=(t == 0),
                            stop=(t == NT - 1),
                            tile_position=(0, 32 * g),
                        )
                if last:
                    yf = ypool.tile([P, GW], mybir.dt.float32, tag="yf")
                    nc.vector.tensor_tensor(
                        yf[:, :], ps[:, :], b_sb[:, :], op=mybir.AluOpType.add
                    )
                    nc.vector.tensor_scalar(
                        yf[96:97, :], yf[96:97, :], float(l_val), float(u_val),
                        mybir.AluOpType.max, mybir.AluOpType.min,
                    )
                    for g in range(NG):
                        nc.sync.dma_start(
                            out_ext[:, g * GW : (g + 1) * GW], yf[32 * g : 32 * g + 1, :]
                        )
                else:
                    yb = ypool.tile([P, GW], mybir.dt.bfloat16, tag="yb")
                    nc.vector.tensor_tensor(
                        yb[:, :], ps[:, :], b_sb[:, :], op=mybir.AluOpType.add
                    )
                    nc.vector.tensor_scalar(
                        yb[96:97, :], yb[96:97, :], float(l_val), float(u_val),
                        mybir.AluOpType.max, mybir.AluOpType.min,
                    )
                    agin = dpool.tile([1, ROWS], mybir.dt.bfloat16, tag="agin")
                    for g in range(NG):
                        nc.sync.dma_start(
                            agin[:, g * GW : (g + 1) * GW], yb[32 * g : 32 * g + 1, :]
                        )
                    agout = dpool.tile([P, NT], mybir.dt.bfloat16, tag="agout")
                    nc.gpsimd.collective_compute(
                        "AllGather",
                        mybir.AluOpType.bypass,
                        replica_groups=[list(range(NCORES))],
                        ins=[agin.opt()],
                        outs=[agout.opt()],
                    )
                    xb = xpool.tile([P, NT], mybir.dt.bfloat16, tag="xb")
                    nc.sync.dma_start(xb[:], agout[:])
    nc.compile()
    return nc


def _get_nc(n_iter, l_val, u_val):
    key = (n_iter, float(l_val), float(u_val))
    if key not in _nc_cache:
        _nc_cache[key] = _build_nc(n_iter, l_val, u_val)
    return _nc_cache[key]


def _prep_in_maps(x, W, b, idx1, idx2):
    perm, _un = _perm(idx1, idx2)
    cm = _colmap()                      # [128, 64] -> flat local-order index
    colidx = perm[cm.reshape(-1)].reshape(P, NT)  # global x index per (p, t)
    bf16 = ml_dtypes.bfloat16
    xp = np.asarray(x, np.float32)
    x0_layout = np.ascontiguousarray(xp[colidx], np.float32)  # [128, 64]
    in_maps = []
    for i in range(NCORES):
        rows_i = perm[ROWS * i : ROWS * (i + 1)]
        Wi = W[rows_i]                   # [1024, 8192] rows in local order
        # Wt[p, (t*4+g)*256 + n] = Wi[g*256+n, colidx[p, t]]
        Wc = Wi[:, colidx.reshape(-1)].reshape(ROWS, P, NT)   # [j, p, t]
        Wt = np.ascontiguousarray(
            np.transpose(Wc, (1, 2, 0)).reshape(P, NT * ROWS)
        ).astype(bf16)                   # Wt[p, t*1024 + j], j = g*256+n
        m = {
            f"W{c}": np.ascontiguousarray(
                Wt[:, c * (NT * ROWS // NWCH) : (c + 1) * (NT * ROWS // NWCH)]
            )
            for c in range(NWCH)
        }
        m["x0"] = x0_layout
        bl = np.asarray(b, np.float32)[rows_i]   # [1024] local order
        bmat = np.zeros((P, GW), np.float32)
        for g in range(NG):
            bmat[32 * g, :] = bl[g * GW : (g + 1) * GW]
        m["bias"] = bmat
        in_maps.append(m)
    return in_maps, perm


def run(x, W, b, l, u, idx1, idx2, N, trace=False, trace_kwargs=None):
    from concourse.bass_utils import run_bass_kernel_spmd

    x = np.asarray(x, np.float32)
    W = np.asarray(W, np.float32)
    b = np.asarray(b, np.float32)
    l = float(np.asarray(l))
    u = float(np.asarray(u))
    idx1 = int(np.asarray(idx1))
    idx2 = int(np.asarray(idx2))
    N = int(np.asarray(N))
    assert x.shape == (D,) and W.shape == (D, D) and b.shape == (D,)
    assert N >= 1

    nc = _get_nc(N, l, u)
    in_maps, perm = _prep_in_maps(x, W, b, idx1, idx2)
    res = run_bass_kernel_spmd(
        nc,
        in_maps,
        core_ids=list(range(NCORES)),
        trace=trace,
        **(trace_kwargs or {}),
    )
    chunks = [np.asarray(res.results[i]["out"], np.float32).reshape(ROWS) for i in range(NCORES)]
    out = np.empty(D, np.float32)
    out[perm] = np.concatenate(chunks)
    return out, res


def kernel(**inputs):
    out, _ = run(
        inputs["x"],
        inputs["W"],
        inputs["b"],
        inputs["l"],
        inputs["u"],
        inputs["idx1"],
        inputs["idx2"],
        inputs["N"],
        trace=False,
    )
    return out
